# revision 10
# baseline (speedup 1.0000x reference)
"""Trainium2 Bass kernel for CollisionDistanceEvaluator (segment_reduce).

Contract: kernel(**inputs) takes FULL inputs (trans [4096,3] f32,
quat [4096,4] f32, pc [4096,4096,3] f32) and returns the FULL output
[4096,1] f32, running the heavy per-point work on 8 NeuronCores
(pure data-parallel over the batch dim, 512 batches/core).

Math: reference rotates pc by inv(quat), translates by -trans, tests an
axis-aligned box, and takes the per-batch masked mean of point norms.
Host precomputes per-batch affine coefficients (O(B) work, like weight
prep):
    R[b]   : rotation matrix of q_inv (no unit-norm assumption)
    A[b]   = R[b] / H[:,None]         (H = box half extents)
    o[b]   = -(trans[b] + C) / H      (C = box center)
so on device, per point p:
    u_i    = A_i . p + o_i            (scaled box coords)
    mask   = max_i |u_i| <= 1
    x'_i   = H_i u_i + C_i            (rotated-translated coords)
    norm   = sqrt(sum_i x'_i^2)
    out[b] = -10000 * sum(mask*norm)/max(cnt,1)   (or +10000 if cnt==0)

Device pipeline per 128-batch tile (batch->partitions, points->free
dim, fp16): 9 DVE ops for the affine (tensor_scalar 4x, tensor_tensor
2x), 3 ACT abs + 2 DVE max for the box test, 3 ACT squares, 2 DVE
adds, is_le + mask*n2, ACT sqrt; masked sum and count ride accum_out.
First and last tiles run in half-width chunks to shrink the pipeline
head (DMA fill) and tail (dependent ACT chain); middle tiles use full
4096-wide ops, which have the best per-op overhead amortization.
"""

import numpy as np

import concourse.bass as bass
import concourse.bacc as bacc
import concourse.mybir as mybir
from concourse.tile import TileContext
from concourse.bass_utils import run_bass_kernel_spmd

def _ensure_ntff_hook():
    """Register the axon NTFF profile hook if the image's antenv lacks it.

    Lets run_bass_kernel_spmd(trace=True) return exec_time_ns under axon.
    Harmless no-op when unavailable.
    """
    import sys
    import types
    try:
        from antenv.axon_hooks import get_axon_ntff_profile_hook  # noqa
        return
    except ImportError:
        pass
    try:
        import antenv
        from trn_agent_boot.trn_boot import _ntff_profile_via_ctypes
        mod = types.ModuleType("antenv.axon_hooks")
        mod._hook = _ntff_profile_via_ctypes("/opt/axon/libaxon_pjrt.so")

        def set_axon_ntff_profile_hook(h):
            mod._hook = h

        def get_axon_ntff_profile_hook():
            return mod._hook

        mod.set_axon_ntff_profile_hook = set_axon_ntff_profile_hook
        mod.get_axon_ntff_profile_hook = get_axon_ntff_profile_hook
        sys.modules["antenv.axon_hooks"] = mod
        antenv.axon_hooks = mod
    except Exception:
        pass


_ensure_ntff_hook()

N_CORES = 8
B_FULL, N_PTS = 4096, 4096
B_CORE = B_FULL // N_CORES          # 512
N_TILES = B_CORE // 128             # 4

DIST_THRESHOLD = 0.001
DIST_COEFF = 10000.0
BOX_CENTER = np.array([-0.001782, 1.005e-05, 0.0431621], dtype=np.float64)
HALF_EXT = np.array([
    0.204416 / 2 + DIST_THRESHOLD,
    0.0632517 / 2 + DIST_THRESHOLD,
    0.1381738 / 2 + DIST_THRESHOLD,
], dtype=np.float64)

_NC_CACHE = {}


def _build_bass():
    f16, f32 = mybir.dt.float16, mybir.dt.float32
    Alu = mybir.AluOpType
    Act = mybir.ActivationFunctionType
    H = HALF_EXT
    C = BOX_CENTER

    nc = bacc.Bacc()
    xyz = nc.declare_dram_parameter(
        "xyz", [3, B_CORE, N_PTS], f16, isOutput=False)
    coef = nc.declare_dram_parameter(
        "coef", [N_TILES, 128, 12], f32, isOutput=False)
    out = nc.declare_dram_parameter("out", [B_CORE, 1], f32, isOutput=True)
    xyz_ap, coef_ap, out_ap = xyz[:], coef[:], out[:]

    with TileContext(nc) as tc, \
            tc.tile_pool(name="data", bufs=2) as data, \
            tc.tile_pool(name="work", bufs=2) as work, \
            tc.tile_pool(name="consts", bufs=1) as consts, \
            tc.tile_pool(name="small", bufs=8) as small:
        # per-partition bias vectors for the ACT squares (bias must be AP)
        cbias = []
        for i in range(3):
            cb = consts.tile([128, 1], f32, tag=f"cb{i}")
            nc.vector.memset(cb[:], float(C[i]))
            cbias.append(cb)
        zbias = consts.tile([128, 1], f32, tag="zb")
        nc.vector.memset(zbias[:], 0.0)

        def chunk(r, ct, cs, fd):
            """Process points cs..cs+fd of the 128 batches in rows r.
            Returns (rc, rs) accumulator tiles [128,1]."""
            xt = data.tile([128, fd], f16, tag="x")
            yt = data.tile([128, fd], f16, tag="y")
            zt = data.tile([128, fd], f16, tag="z")
            col = slice(cs, cs + fd)
            nc.sync.dma_start(out=xt[:], in_=xyz_ap[0, r, col])
            nc.sync.dma_start(out=yt[:], in_=xyz_ap[1, r, col])
            nc.sync.dma_start(out=zt[:], in_=xyz_ap[2, r, col])

            # u_i = A_i0*x + A_i1*y + A_i2*z + o_i
            # one z-product rides ACT (scale accepts a per-partition AP)
            # to shave the Vector bottleneck
            u = []
            for i in range(3):
                ui = work.tile([128, fd], f16, tag=f"u{i}")
                qy = work.tile([128, fd], f16, tag="qy")
                qz = work.tile([128, fd], f16, tag="qz")
                nc.vector.tensor_scalar(
                    ui[:], xt[:],
                    ct[:, 3 * i:3 * i + 1], ct[:, 9 + i:10 + i],
                    Alu.mult, Alu.add)
                nc.vector.tensor_scalar(
                    qy[:], yt[:], ct[:, 3 * i + 1:3 * i + 2], None,
                    Alu.mult)
                if i == 2:
                    nc.scalar.activation(
                        qz[:], zt[:], Act.Identity, bias=zbias[:],
                        scale=ct[:, 3 * i + 2:3 * i + 3])
                else:
                    nc.vector.tensor_scalar(
                        qz[:], zt[:], ct[:, 3 * i + 2:3 * i + 3], None,
                        Alu.mult)
                nc.vector.tensor_tensor(ui[:], ui[:], qy[:], Alu.add)
                nc.vector.tensor_tensor(ui[:], ui[:], qz[:], Alu.add)
                u.append(ui)

            # squares for the norm: w_i = (H_i*u_i + C_i)^2 (ACT), into
            # the consumed x/y/z buffers
            w = [xt, yt, zt]
            for i in range(3):
                nc.scalar.activation(
                    w[i][:], u[i][:], Act.Square,
                    bias=cbias[i][:], scale=float(H[i]))

            # box test: mx = max_i |u_i|; all three abs on ACT (it has
            # slack; DVE is the bottleneck), max-combine on DVE
            for i in range(3):
                nc.scalar.activation(u[i][:], u[i][:], Act.Abs,
                                     bias=zbias[:])
            mx = work.tile([128, fd], f16, tag="mx")
            nc.vector.tensor_tensor(mx[:], u[0][:], u[1][:], Alu.max)
            nc.vector.tensor_tensor(mx[:], mx[:], u[2][:], Alu.max)

            # n2 = w0 + w1 + w2 (in place into w0 = xt buffer)
            n2 = w[0]
            nc.vector.tensor_tensor(n2[:], w[0][:], w[1][:], Alu.add)
            nc.vector.tensor_tensor(n2[:], n2[:], w[2][:], Alu.add)

            # mask = (mx <= 1); mm = mask * n2; count on ACT (accum free)
            mask = work.tile([128, fd], f16, tag="mask")
            nc.vector.tensor_scalar(mask[:], mx[:], 1.0, None, Alu.is_le)
            mm = mx  # mx consumed; reuse its buffer
            nc.vector.tensor_tensor(mm[:], mask[:], n2[:], Alu.mult)
            rc = small.tile([128, 1], f32, tag="rc")
            nc.scalar.activation(
                qy[:], mask[:], Act.Identity, bias=zbias[:],
                accum_out=rc[:])

            # masked norms + fused row-sum (qz is a dead scratch output)
            rs = small.tile([128, 1], f32, tag="rs")
            nc.scalar.activation(qz[:], mm[:], Act.Sqrt, accum_out=rs[:])
            return rc, rs

        for t in range(N_TILES):
            r = slice(t * 128, (t + 1) * 128)
            ct = small.tile([128, 12], f32, tag="coef")
            nc.sync.dma_start(out=ct[:], in_=coef_ap[t, :, :])

            # first tile: ascending chunks so compute starts after a
            # small DMA; last tile: descending so the tail chain is short
            if t == 0:
                fds = [1024, 1024, 2048]
            elif t == N_TILES - 1:
                fds = [2048, 1024, 1024]
            else:
                fds = [N_PTS]
            accs = []
            cs = 0
            for fd in fds:
                accs.append(chunk(r, ct, cs, fd))
                cs += fd
            rc, rs = accs[0]
            for rc2, rs2 in accs[1:]:
                nc.vector.tensor_tensor(rc[:], rc[:], rc2[:], Alu.add)
                nc.vector.tensor_tensor(rs[:], rs[:], rs2[:], Alu.add)

            # out = (cnt==0)*10000 + (-10000*rs)/max(cnt,1)
            rc1 = small.tile([128, 1], f32, tag="rc1")
            nc.vector.tensor_scalar(rc1[:], rc[:], 1.0, None, Alu.max)
            inv = small.tile([128, 1], f32, tag="inv")
            nc.vector.reciprocal(inv[:], rc1[:])
            val = small.tile([128, 1], f32, tag="val")
            nc.vector.scalar_tensor_tensor(
                val[:], rs[:], -DIST_COEFF, inv[:], Alu.mult, Alu.mult)
            zer = small.tile([128, 1], f32, tag="zer")
            nc.vector.tensor_scalar(zer[:], rc[:], 0.0, None, Alu.is_le)
            ot = small.tile([128, 1], f32, tag="ot")
            nc.vector.scalar_tensor_tensor(
                ot[:], zer[:], DIST_COEFF, val[:], Alu.mult, Alu.add)
            nc.sync.dma_start(out=out_ap[r, :], in_=ot[:])
    nc.compile()
    return nc


def _get_nc():
    if "nc" not in _NC_CACHE:
        _NC_CACHE["nc"] = _build_bass()
    return _NC_CACHE["nc"]


def _host_coefficients(trans, quat):
    """Per-batch A [B,3,3] and o [B,3] in f32 (computed in f64)."""
    q = np.asarray(quat, np.float64)
    t = np.asarray(trans, np.float64)
    B = q.shape[0]
    s = (q * q).sum(-1)
    qi = np.concatenate([-q[:, :3], q[:, 3:]], -1) / s[:, None]
    v, w = qi[:, :3], qi[:, 3]
    vv = v[:, :, None] * v[:, None, :]
    w2mv = w * w - (v * v).sum(-1)
    Vx = np.zeros((B, 3, 3))
    Vx[:, 0, 1] = -v[:, 2]
    Vx[:, 0, 2] = v[:, 1]
    Vx[:, 1, 0] = v[:, 2]
    Vx[:, 1, 2] = -v[:, 0]
    Vx[:, 2, 0] = -v[:, 1]
    Vx[:, 2, 1] = v[:, 0]
    R = (w2mv[:, None, None] * np.eye(3)
         + 2.0 * vv
         + 2.0 * w[:, None, None] * Vx)
    A = R / HALF_EXT[None, :, None]
    o = -(t + BOX_CENTER[None, :]) / HALF_EXT[None, :]
    return A.astype(np.float32), o.astype(np.float32)


def _make_in_maps(trans, quat, pc):
    A, o = _host_coefficients(trans, quat)
    coef_full = np.concatenate(
        [A.reshape(B_FULL, 9), o], axis=1).astype(np.float32)  # [B,12]
    # planar fp16 [3, B, N]
    pcT = np.ascontiguousarray(
        np.asarray(pc, np.float32).transpose(2, 0, 1)).astype(np.float16)
    in_maps = []
    for c in range(N_CORES):
        bs, be = c * B_CORE, (c + 1) * B_CORE
        in_maps.append({
            "xyz": np.ascontiguousarray(pcT[:, bs:be, :]),
            "coef": np.ascontiguousarray(
                coef_full[bs:be].reshape(N_TILES, 128, 12)),
        })
    return in_maps


def run_spmd(trans, quat, pc, **spmd_kwargs):
    """Shard, run on 8 cores, gather. Returns (output, BassKernelResults)."""
    in_maps = _make_in_maps(trans, quat, pc)
    res = run_bass_kernel_spmd(
        _get_nc(), in_maps, list(range(N_CORES)), **spmd_kwargs)
    outs = [res.results[i]["out"] for i in range(N_CORES)]
    full = np.concatenate(outs, axis=0).astype(np.float32)
    return full, res


def kernel(trans, quat, pc):
    full, _ = run_spmd(trans, quat, pc)
    return full


# revision 12
# speedup vs baseline: 1.0561x; 1.0561x over previous
"""Trainium2 Bass kernel for CollisionDistanceEvaluator (segment_reduce).

Contract: kernel(**inputs) takes FULL inputs (trans [4096,3] f32,
quat [4096,4] f32, pc [4096,4096,3] f32) and returns the FULL output
[4096,1] f32, running the heavy per-point work on 8 NeuronCores
(pure data-parallel over the batch dim, 512 batches/core).

Math: reference rotates pc by inv(quat), translates by -trans, tests an
axis-aligned box, and takes the per-batch masked mean of point norms.
Host precomputes per-batch affine coefficients (O(B) work, like weight
prep):
    R[b]   : rotation matrix of q_inv (no unit-norm assumption)
    A[b]   = R[b] / H[:,None]         (H = box half extents)
    o[b]   = -(trans[b] + C) / H      (C = box center)
so on device, per point p:
    u_i    = A_i . p + o_i            (scaled box coords)
    mask   = max_i |u_i| <= 1
    x'_i   = H_i u_i + C_i            (rotated-translated coords)
    norm   = sqrt(sum_i x'_i^2)
    out[b] = -10000 * sum(mask*norm)/max(cnt,1)   (or +10000 if cnt==0)

Device pipeline per 128-batch tile (batch->partitions, points->free
dim, fp16): 9 DVE ops for the affine (tensor_scalar 4x, tensor_tensor
2x), 3 ACT abs + 2 DVE max for the box test, 3 ACT squares, 2 DVE
adds, is_le + mask*n2, ACT sqrt; masked sum and count ride accum_out.
First and last tiles run in half-width chunks to shrink the pipeline
head (DMA fill) and tail (dependent ACT chain); middle tiles use full
4096-wide ops, which have the best per-op overhead amortization.
"""

import numpy as np

import concourse.bass as bass
import concourse.bacc as bacc
import concourse.mybir as mybir
from concourse.tile import TileContext
from concourse.bass_utils import run_bass_kernel_spmd

def _ensure_ntff_hook():
    """Register the axon NTFF profile hook if the image's antenv lacks it.

    Lets run_bass_kernel_spmd(trace=True) return exec_time_ns under axon.
    Harmless no-op when unavailable.
    """
    import sys
    import types
    try:
        from antenv.axon_hooks import get_axon_ntff_profile_hook  # noqa
        return
    except ImportError:
        pass
    try:
        import antenv
        from trn_agent_boot.trn_boot import _ntff_profile_via_ctypes
        mod = types.ModuleType("antenv.axon_hooks")
        mod._hook = _ntff_profile_via_ctypes("/opt/axon/libaxon_pjrt.so")

        def set_axon_ntff_profile_hook(h):
            mod._hook = h

        def get_axon_ntff_profile_hook():
            return mod._hook

        mod.set_axon_ntff_profile_hook = set_axon_ntff_profile_hook
        mod.get_axon_ntff_profile_hook = get_axon_ntff_profile_hook
        sys.modules["antenv.axon_hooks"] = mod
        antenv.axon_hooks = mod
    except Exception:
        pass


_ensure_ntff_hook()

N_CORES = 8
B_FULL, N_PTS = 4096, 4096
B_CORE = B_FULL // N_CORES          # 512
N_TILES = B_CORE // 128             # 4

DIST_THRESHOLD = 0.001
DIST_COEFF = 10000.0
BOX_CENTER = np.array([-0.001782, 1.005e-05, 0.0431621], dtype=np.float64)
HALF_EXT = np.array([
    0.204416 / 2 + DIST_THRESHOLD,
    0.0632517 / 2 + DIST_THRESHOLD,
    0.1381738 / 2 + DIST_THRESHOLD,
], dtype=np.float64)

_NC_CACHE = {}


def _build_bass():
    f16, f32 = mybir.dt.float16, mybir.dt.float32
    Alu = mybir.AluOpType
    Act = mybir.ActivationFunctionType
    H = HALF_EXT
    C = BOX_CENTER

    nc = bacc.Bacc()
    xyz = nc.declare_dram_parameter(
        "xyz", [3, B_CORE, N_PTS], f16, isOutput=False)
    coef = nc.declare_dram_parameter(
        "coef", [N_TILES, 128, 12], f32, isOutput=False)
    out = nc.declare_dram_parameter("out", [B_CORE, 1], f32, isOutput=True)
    xyz_ap, coef_ap, out_ap = xyz[:], coef[:], out[:]

    with TileContext(nc) as tc, \
            tc.tile_pool(name="data", bufs=2) as data, \
            tc.tile_pool(name="work", bufs=2) as work, \
            tc.tile_pool(name="consts", bufs=1) as consts, \
            tc.tile_pool(name="small", bufs=8) as small:
        # per-partition bias vectors for the ACT squares (bias must be AP)
        cbias = []
        for i in range(3):
            cb = consts.tile([128, 1], f32, tag=f"cb{i}")
            nc.vector.memset(cb[:], float(C[i]))
            cbias.append(cb)
        zbias = consts.tile([128, 1], f32, tag="zb")
        nc.vector.memset(zbias[:], 0.0)

        def chunk(r, ct, cs, fd):
            """Process points cs..cs+fd of the 128 batches in rows r.
            Returns (rc, rs) accumulator tiles [128,1]."""
            xt = data.tile([128, fd], f16, tag="x")
            yt = data.tile([128, fd], f16, tag="y")
            zt = data.tile([128, fd], f16, tag="z")
            col = slice(cs, cs + fd)
            nc.sync.dma_start(out=xt[:], in_=xyz_ap[0, r, col])
            nc.sync.dma_start(out=yt[:], in_=xyz_ap[1, r, col])
            nc.sync.dma_start(out=zt[:], in_=xyz_ap[2, r, col])

            # u_i = A_i0*x + A_i1*y + A_i2*z + o_i
            # one z-product rides ACT (scale accepts a per-partition AP)
            # to shave the Vector bottleneck
            u = []
            for i in range(3):
                ui = work.tile([128, fd], f16, tag=f"u{i}")
                qy = work.tile([128, fd], f16, tag="qy")
                qz = work.tile([128, fd], f16, tag="qz")
                nc.vector.tensor_scalar(
                    ui[:], xt[:],
                    ct[:, 3 * i:3 * i + 1], ct[:, 9 + i:10 + i],
                    Alu.mult, Alu.add)
                nc.vector.tensor_scalar(
                    qy[:], yt[:], ct[:, 3 * i + 1:3 * i + 2], None,
                    Alu.mult)
                nc.vector.tensor_scalar(
                    qz[:], zt[:], ct[:, 3 * i + 2:3 * i + 3], None,
                    Alu.mult)
                nc.vector.tensor_tensor(ui[:], ui[:], qy[:], Alu.add)
                nc.vector.tensor_tensor(ui[:], ui[:], qz[:], Alu.add)
                u.append(ui)

            # squares for the norm: w_i = (H_i*u_i + C_i)^2 (ACT), into
            # the consumed x/y/z buffers
            w = [xt, yt, zt]
            for i in range(3):
                nc.scalar.activation(
                    w[i][:], u[i][:], Act.Square,
                    bias=cbias[i][:], scale=float(H[i]))

            # box test: mx = max_i |u_i|; all three abs on ACT (it has
            # slack; DVE is the bottleneck), max-combine on DVE
            for i in range(3):
                nc.scalar.activation(u[i][:], u[i][:], Act.Abs,
                                     bias=zbias[:])
            mx = work.tile([128, fd], f16, tag="mx")
            nc.vector.tensor_tensor(mx[:], u[0][:], u[1][:], Alu.max)
            nc.vector.tensor_tensor(mx[:], mx[:], u[2][:], Alu.max)

            # n2 = w0 + w1 + w2 (in place into w0 = xt buffer)
            n2 = w[0]
            nc.vector.tensor_tensor(n2[:], w[0][:], w[1][:], Alu.add)
            nc.vector.tensor_tensor(n2[:], n2[:], w[2][:], Alu.add)

            # mask = (mx <= 1); mm = mask * n2; count on ACT (accum free)
            mask = work.tile([128, fd], f16, tag="mask")
            nc.vector.tensor_scalar(mask[:], mx[:], 1.0, None, Alu.is_le)
            mm = mx  # mx consumed; reuse its buffer
            nc.vector.tensor_tensor(mm[:], mask[:], n2[:], Alu.mult)
            rc = small.tile([128, 1], f32, tag="rc")
            nc.scalar.activation(
                qy[:], mask[:], Act.Identity, bias=zbias[:],
                accum_out=rc[:])

            # masked norms + fused row-sum (qz is a dead scratch output)
            rs = small.tile([128, 1], f32, tag="rs")
            nc.scalar.activation(qz[:], mm[:], Act.Sqrt, accum_out=rs[:])
            return rc, rs

        # hoist all (tiny) coef DMAs so they never gate first compute
        cts = []
        for t in range(N_TILES):
            ct = small.tile([128, 12], f32, tag=f"coef{t}")
            nc.sync.dma_start(out=ct[:], in_=coef_ap[t, :, :])
            cts.append(ct)
        for t in range(N_TILES):
            r = slice(t * 128, (t + 1) * 128)
            ct = cts[t]

            # first tile: ascending chunks so compute starts after a
            # small DMA; last tile: descending so the tail chain is short
            if t == 0:
                fds = [1024, 1024, 2048]
            elif t == N_TILES - 1:
                fds = [2048, 1024, 1024]
            else:
                fds = [N_PTS]
            accs = []
            cs = 0
            for fd in fds:
                accs.append(chunk(r, ct, cs, fd))
                cs += fd
            rc, rs = accs[0]
            for rc2, rs2 in accs[1:]:
                nc.vector.tensor_tensor(rc[:], rc[:], rc2[:], Alu.add)
                nc.vector.tensor_tensor(rs[:], rs[:], rs2[:], Alu.add)

            # out = (cnt==0)*10000 + (-10000*rs)/max(cnt,1)
            rc1 = small.tile([128, 1], f32, tag="rc1")
            nc.vector.tensor_scalar(rc1[:], rc[:], 1.0, None, Alu.max)
            inv = small.tile([128, 1], f32, tag="inv")
            nc.vector.reciprocal(inv[:], rc1[:])
            val = small.tile([128, 1], f32, tag="val")
            nc.vector.scalar_tensor_tensor(
                val[:], rs[:], -DIST_COEFF, inv[:], Alu.mult, Alu.mult)
            zer = small.tile([128, 1], f32, tag="zer")
            nc.vector.tensor_scalar(zer[:], rc[:], 0.0, None, Alu.is_le)
            ot = small.tile([128, 1], f32, tag="ot")
            nc.vector.scalar_tensor_tensor(
                ot[:], zer[:], DIST_COEFF, val[:], Alu.mult, Alu.add)
            nc.sync.dma_start(out=out_ap[r, :], in_=ot[:])
    nc.compile()
    return nc


def _get_nc():
    if "nc" not in _NC_CACHE:
        _NC_CACHE["nc"] = _build_bass()
    return _NC_CACHE["nc"]


def _host_coefficients(trans, quat):
    """Per-batch A [B,3,3] and o [B,3] in f32 (computed in f64)."""
    q = np.asarray(quat, np.float64)
    t = np.asarray(trans, np.float64)
    B = q.shape[0]
    s = (q * q).sum(-1)
    qi = np.concatenate([-q[:, :3], q[:, 3:]], -1) / s[:, None]
    v, w = qi[:, :3], qi[:, 3]
    vv = v[:, :, None] * v[:, None, :]
    w2mv = w * w - (v * v).sum(-1)
    Vx = np.zeros((B, 3, 3))
    Vx[:, 0, 1] = -v[:, 2]
    Vx[:, 0, 2] = v[:, 1]
    Vx[:, 1, 0] = v[:, 2]
    Vx[:, 1, 2] = -v[:, 0]
    Vx[:, 2, 0] = -v[:, 1]
    Vx[:, 2, 1] = v[:, 0]
    R = (w2mv[:, None, None] * np.eye(3)
         + 2.0 * vv
         + 2.0 * w[:, None, None] * Vx)
    A = R / HALF_EXT[None, :, None]
    o = -(t + BOX_CENTER[None, :]) / HALF_EXT[None, :]
    return A.astype(np.float32), o.astype(np.float32)


def _make_in_maps(trans, quat, pc):
    A, o = _host_coefficients(trans, quat)
    coef_full = np.concatenate(
        [A.reshape(B_FULL, 9), o], axis=1).astype(np.float32)  # [B,12]
    # planar fp16 [3, B, N]
    pcT = np.ascontiguousarray(
        np.asarray(pc, np.float32).transpose(2, 0, 1)).astype(np.float16)
    in_maps = []
    for c in range(N_CORES):
        bs, be = c * B_CORE, (c + 1) * B_CORE
        in_maps.append({
            "xyz": np.ascontiguousarray(pcT[:, bs:be, :]),
            "coef": np.ascontiguousarray(
                coef_full[bs:be].reshape(N_TILES, 128, 12)),
        })
    return in_maps


def run_spmd(trans, quat, pc, **spmd_kwargs):
    """Shard, run on 8 cores, gather. Returns (output, BassKernelResults)."""
    in_maps = _make_in_maps(trans, quat, pc)
    res = run_bass_kernel_spmd(
        _get_nc(), in_maps, list(range(N_CORES)), **spmd_kwargs)
    outs = [res.results[i]["out"] for i in range(N_CORES)]
    full = np.concatenate(outs, axis=0).astype(np.float32)
    return full, res


def kernel(trans, quat, pc):
    full, _ = run_spmd(trans, quat, pc)
    return full


# revision 13
# speedup vs baseline: 1.1100x; 1.0509x over previous
"""Trainium2 Bass kernel for CollisionDistanceEvaluator (segment_reduce).

Contract: kernel(**inputs) takes FULL inputs (trans [4096,3] f32,
quat [4096,4] f32, pc [4096,4096,3] f32) and returns the FULL output
[4096,1] f32, running the heavy per-point work on 8 NeuronCores
(pure data-parallel over the batch dim, 512 batches/core).

Math: reference rotates pc by inv(quat), translates by -trans, tests an
axis-aligned box, and takes the per-batch masked mean of point norms.
Host precomputes per-batch affine coefficients (O(B) work, like weight
prep):
    R[b]   : rotation matrix of q_inv (no unit-norm assumption)
    A[b]   = R[b] / H[:,None]         (H = box half extents)
    o[b]   = -(trans[b] + C) / H      (C = box center)
so on device, per point p:
    u_i    = A_i . p + o_i            (scaled box coords)
    mask   = max_i |u_i| <= 1
    x'_i   = H_i u_i + C_i            (rotated-translated coords)
    norm   = sqrt(sum_i x'_i^2)
    out[b] = -10000 * sum(mask*norm)/max(cnt,1)   (or +10000 if cnt==0)

Device pipeline per 128-batch tile (batch->partitions, points->free
dim, fp16). Engines execute their instruction streams in order, so the
emission is software-pipelined in two stages per chunk:
  stage A: DMA + affine (9 tensor_scalar @4x + 6 tensor_tensor @2x on
           DVE), then 3 squares + 3 abs on ACT
  stage B: box max-combine, n2 sum, mask, mask*n2 on DVE; count +
           sqrt-with-accum on ACT
Stage B of chunk c is emitted after stage A of chunk c+1, so the DVE
never sits waiting for ACT's abs results — it runs the next chunk's
affine instead. First/last tiles use half-width chunks to shrink the
DMA-fill head and the dependent tail.
"""

import numpy as np

import concourse.bass as bass
import concourse.bacc as bacc
import concourse.mybir as mybir
from concourse.tile import TileContext
from concourse.bass_utils import run_bass_kernel_spmd

def _ensure_ntff_hook():
    """Register the axon NTFF profile hook if the image's antenv lacks it.

    Lets run_bass_kernel_spmd(trace=True) return exec_time_ns under axon.
    Harmless no-op when unavailable.
    """
    import sys
    import types
    try:
        from antenv.axon_hooks import get_axon_ntff_profile_hook  # noqa
        return
    except ImportError:
        pass
    try:
        import antenv
        from trn_agent_boot.trn_boot import _ntff_profile_via_ctypes
        mod = types.ModuleType("antenv.axon_hooks")
        mod._hook = _ntff_profile_via_ctypes("/opt/axon/libaxon_pjrt.so")

        def set_axon_ntff_profile_hook(h):
            mod._hook = h

        def get_axon_ntff_profile_hook():
            return mod._hook

        mod.set_axon_ntff_profile_hook = set_axon_ntff_profile_hook
        mod.get_axon_ntff_profile_hook = get_axon_ntff_profile_hook
        sys.modules["antenv.axon_hooks"] = mod
        antenv.axon_hooks = mod
    except Exception:
        pass


_ensure_ntff_hook()

N_CORES = 8
B_FULL, N_PTS = 4096, 4096
B_CORE = B_FULL // N_CORES          # 512
N_TILES = B_CORE // 128             # 4

DIST_THRESHOLD = 0.001
DIST_COEFF = 10000.0
BOX_CENTER = np.array([-0.001782, 1.005e-05, 0.0431621], dtype=np.float64)
HALF_EXT = np.array([
    0.204416 / 2 + DIST_THRESHOLD,
    0.0632517 / 2 + DIST_THRESHOLD,
    0.1381738 / 2 + DIST_THRESHOLD,
], dtype=np.float64)

_NC_CACHE = {}


def _build_bass():
    f16, f32 = mybir.dt.float16, mybir.dt.float32
    Alu = mybir.AluOpType
    Act = mybir.ActivationFunctionType
    H = HALF_EXT
    C = BOX_CENTER

    nc = bacc.Bacc()
    xyz = nc.declare_dram_parameter(
        "xyz", [3, B_CORE, N_PTS], f16, isOutput=False)
    coef = nc.declare_dram_parameter(
        "coef", [N_TILES, 128, 12], f32, isOutput=False)
    out = nc.declare_dram_parameter("out", [B_CORE, 1], f32, isOutput=True)
    xyz_ap, coef_ap, out_ap = xyz[:], coef[:], out[:]

    with TileContext(nc) as tc, \
            tc.tile_pool(name="data", bufs=3) as data, \
            tc.tile_pool(name="work", bufs=2) as work, \
            tc.tile_pool(name="consts", bufs=1) as consts, \
            tc.tile_pool(name="small", bufs=8) as small:
        # per-partition bias vectors for the ACT squares (bias must be AP)
        cbias = []
        for i in range(3):
            cb = consts.tile([128, 1], f32, tag=f"cb{i}")
            nc.vector.memset(cb[:], float(C[i]))
            cbias.append(cb)
        zbias = consts.tile([128, 1], f32, tag="zb")
        nc.vector.memset(zbias[:], 0.0)

        # hoist all (tiny) coef DMAs so they never gate first compute
        cts = []
        for t in range(N_TILES):
            ctt = small.tile([128, 12], f32, tag=f"coef{t}")
            nc.sync.dma_start(out=ctt[:], in_=coef_ap[t, :, :])
            cts.append(ctt)

        def stage_a(t, cs, fd):
            """DMA + affine (DVE) + squares/abs (ACT). Returns state."""
            r = slice(t * 128, (t + 1) * 128)
            ct = cts[t]
            xt = data.tile([128, fd], f16, tag="x")
            yt = data.tile([128, fd], f16, tag="y")
            zt = data.tile([128, fd], f16, tag="z")
            col = slice(cs, cs + fd)
            nc.sync.dma_start(out=xt[:], in_=xyz_ap[0, r, col])
            nc.sync.dma_start(out=yt[:], in_=xyz_ap[1, r, col])
            nc.sync.dma_start(out=zt[:], in_=xyz_ap[2, r, col])

            # u_i = A_i0*x + A_i1*y + A_i2*z + o_i
            u = []
            for i in range(3):
                ui = work.tile([128, fd], f16, tag=f"u{i}")
                qy = work.tile([128, fd], f16, tag="qy")
                qz = work.tile([128, fd], f16, tag="qz")
                nc.vector.tensor_scalar(
                    ui[:], xt[:],
                    ct[:, 3 * i:3 * i + 1], ct[:, 9 + i:10 + i],
                    Alu.mult, Alu.add)
                nc.vector.tensor_scalar(
                    qy[:], yt[:], ct[:, 3 * i + 1:3 * i + 2], None,
                    Alu.mult)
                nc.vector.tensor_scalar(
                    qz[:], zt[:], ct[:, 3 * i + 2:3 * i + 3], None,
                    Alu.mult)
                nc.vector.tensor_tensor(ui[:], ui[:], qy[:], Alu.add)
                nc.vector.tensor_tensor(ui[:], ui[:], qz[:], Alu.add)
                u.append(ui)

            # squares for the norm: w_i = (H_i*u_i + C_i)^2 (ACT), into
            # the consumed x/y/z buffers; then in-place |u_i| (ACT)
            w = [xt, yt, zt]
            for i in range(3):
                nc.scalar.activation(
                    w[i][:], u[i][:], Act.Square,
                    bias=cbias[i][:], scale=float(H[i]))
            for i in range(3):
                nc.scalar.activation(u[i][:], u[i][:], Act.Abs,
                                     bias=zbias[:])
            return u, w, fd

        def stage_b(state):
            """Box combine + n2 + mask + mm (DVE); count + sqrt (ACT)."""
            u, w, fd = state
            mx = work.tile([128, fd], f16, tag="mx")
            nc.vector.tensor_tensor(mx[:], u[0][:], u[1][:], Alu.max)
            nc.vector.tensor_tensor(mx[:], mx[:], u[2][:], Alu.max)

            n2 = w[0]
            nc.vector.tensor_tensor(n2[:], w[0][:], w[1][:], Alu.add)
            nc.vector.tensor_tensor(n2[:], n2[:], w[2][:], Alu.add)

            mask = work.tile([128, fd], f16, tag="mask")
            nc.vector.tensor_scalar(mask[:], mx[:], 1.0, None, Alu.is_le)
            mm = mx  # mx consumed; reuse its buffer
            nc.vector.tensor_tensor(mm[:], mask[:], n2[:], Alu.mult)

            rc = small.tile([128, 1], f32, tag="rc")
            nc.scalar.activation(
                mask[:], mask[:], Act.Identity, bias=zbias[:],
                accum_out=rc[:])
            rs = small.tile([128, 1], f32, tag="rs")
            nc.scalar.activation(mm[:], mm[:], Act.Sqrt, accum_out=rs[:])
            return rc, rs

        def tile_tail(t, accs):
            """Combine chunk accumulators, final arithmetic, out DMA."""
            r = slice(t * 128, (t + 1) * 128)
            rc, rs = accs[0]
            for rc2, rs2 in accs[1:]:
                nc.vector.tensor_tensor(rc[:], rc[:], rc2[:], Alu.add)
                nc.vector.tensor_tensor(rs[:], rs[:], rs2[:], Alu.add)
            # out = (cnt==0)*10000 + (-10000*rs)/max(cnt,1)
            rc1 = small.tile([128, 1], f32, tag="rc1")
            nc.vector.tensor_scalar(rc1[:], rc[:], 1.0, None, Alu.max)
            inv = small.tile([128, 1], f32, tag="inv")
            nc.vector.reciprocal(inv[:], rc1[:])
            val = small.tile([128, 1], f32, tag="val")
            nc.vector.scalar_tensor_tensor(
                val[:], rs[:], -DIST_COEFF, inv[:], Alu.mult, Alu.mult)
            zer = small.tile([128, 1], f32, tag="zer")
            nc.vector.tensor_scalar(zer[:], rc[:], 0.0, None, Alu.is_le)
            ot = small.tile([128, 1], f32, tag="ot")
            nc.vector.scalar_tensor_tensor(
                ot[:], zer[:], DIST_COEFF, val[:], Alu.mult, Alu.add)
            nc.sync.dma_start(out=out_ap[r, :], in_=ot[:])

        # chunk list: (tile, col_start, width); first/last tiles halved
        chunks = []
        for t in range(N_TILES):
            if t == 0 or t == N_TILES - 1:
                fds = [N_PTS // 2, N_PTS // 2]
            else:
                fds = [N_PTS]
            cs = 0
            for fd in fds:
                chunks.append((t, cs, fd, fd == fds[-1] and cs + fd == N_PTS))
                cs += fd

        # software-pipelined emission: stage A of chunk k+1 goes out
        # before stage B of chunk k
        pending = None          # (state, tile, is_tile_last)
        accs = []               # per-tile accumulator list
        acc_by_tile = {t: [] for t in range(N_TILES)}
        for (t, cs, fd, last_of_tile) in chunks:
            st = stage_a(t, cs, fd)
            if pending is not None:
                p_state, p_t, p_last = pending
                acc_by_tile[p_t].append(stage_b(p_state))
                if p_last:
                    tile_tail(p_t, acc_by_tile[p_t])
            pending = (st, t, last_of_tile)
        p_state, p_t, p_last = pending
        acc_by_tile[p_t].append(stage_b(p_state))
        if p_last:
            tile_tail(p_t, acc_by_tile[p_t])
    nc.compile()
    return nc


def _get_nc():
    if "nc" not in _NC_CACHE:
        _NC_CACHE["nc"] = _build_bass()
    return _NC_CACHE["nc"]


def _host_coefficients(trans, quat):
    """Per-batch A [B,3,3] and o [B,3] in f32 (computed in f64)."""
    q = np.asarray(quat, np.float64)
    t = np.asarray(trans, np.float64)
    B = q.shape[0]
    s = (q * q).sum(-1)
    qi = np.concatenate([-q[:, :3], q[:, 3:]], -1) / s[:, None]
    v, w = qi[:, :3], qi[:, 3]
    vv = v[:, :, None] * v[:, None, :]
    w2mv = w * w - (v * v).sum(-1)
    Vx = np.zeros((B, 3, 3))
    Vx[:, 0, 1] = -v[:, 2]
    Vx[:, 0, 2] = v[:, 1]
    Vx[:, 1, 0] = v[:, 2]
    Vx[:, 1, 2] = -v[:, 0]
    Vx[:, 2, 0] = -v[:, 1]
    Vx[:, 2, 1] = v[:, 0]
    R = (w2mv[:, None, None] * np.eye(3)
         + 2.0 * vv
         + 2.0 * w[:, None, None] * Vx)
    A = R / HALF_EXT[None, :, None]
    o = -(t + BOX_CENTER[None, :]) / HALF_EXT[None, :]
    return A.astype(np.float32), o.astype(np.float32)


def _make_in_maps(trans, quat, pc):
    A, o = _host_coefficients(trans, quat)
    coef_full = np.concatenate(
        [A.reshape(B_FULL, 9), o], axis=1).astype(np.float32)  # [B,12]
    # planar fp16 [3, B, N]
    pcT = np.ascontiguousarray(
        np.asarray(pc, np.float32).transpose(2, 0, 1)).astype(np.float16)
    in_maps = []
    for c in range(N_CORES):
        bs, be = c * B_CORE, (c + 1) * B_CORE
        in_maps.append({
            "xyz": np.ascontiguousarray(pcT[:, bs:be, :]),
            "coef": np.ascontiguousarray(
                coef_full[bs:be].reshape(N_TILES, 128, 12)),
        })
    return in_maps


def run_spmd(trans, quat, pc, **spmd_kwargs):
    """Shard, run on 8 cores, gather. Returns (output, BassKernelResults)."""
    in_maps = _make_in_maps(trans, quat, pc)
    res = run_bass_kernel_spmd(
        _get_nc(), in_maps, list(range(N_CORES)), **spmd_kwargs)
    outs = [res.results[i]["out"] for i in range(N_CORES)]
    full = np.concatenate(outs, axis=0).astype(np.float32)
    return full, res


def kernel(trans, quat, pc):
    full, _ = run_spmd(trans, quat, pc)
    return full
